# revision 1
# baseline (speedup 1.0000x reference)
"""CFConv-Angular (SchNet triplet message passing) on 8 Trainium2 NeuronCores.

Math (per batch b, atom a, feature f):
    H   = r_ij @ Wf1 + bf1                      [T, G]   (G = n_f = 128)
    S0  = softplus(H)                           [T, G]
    W   = (S0 - log2) @ Wf2 + bf2               [T, F]
    y   = x @ Win                               [A, F]
    P   = y[j] * y[k] * mask                    [T, F]
    out_pre = sum_t P * W                       [F]
    out = softplus(out_pre @ Wout + bout) - log2

Reformulation used on-device (avoids materializing W):
    out_pre[f] = sum_g Wf2[g,f] * C[g,f] + beta[f] * PS[f]
    C[g,f]  = sum_t S0[t,g] * P[t,f]     <- true matmul, contract over t
    PS[f]   = sum_t P[t,f]               <- extra matmul col with ones rhs
    beta    = bf2 - log2 * colsum(Wf2)   <- host constant

Sharding: data-parallel over the 1024 (b, a) pairs, 128 per core.  The
per-batch y table (512 rows x 128 f, fp16) is computed on each core and
written to a DRAM scratch; neighbor rows are fetched with dma_gather
(256B rows).  Filter weights are tiny and replicated.
"""

import os
import sys
from contextlib import ExitStack

import numpy as np

for _p in ("/opt/trn_rl_repo", "/root/.axon_site/_ro/trn_rl_repo"):
    if os.path.isdir(_p) and _p not in sys.path:
        sys.path.append(_p)

B, A, T, NRBF, F = 2, 512, 512, 25, 128
CORES = 8
GA = 8                       # atoms per gather group
PAIRS_PER_CORE = B * A // CORES      # 128
NG_FULL = PAIRS_PER_CORE // GA       # 16 groups per core
NGG = A // GA                        # 64 groups per batch
LOG2 = float(np.log(2.0))

_programs = {}


def _build(ng, general_mask, dbg=False):
    """Build + compile the per-core Bass program covering `ng` groups."""
    import concourse.bacc as bacc
    import concourse.bass as bass
    import concourse.tile as tile
    from concourse import mybir

    dt = mybir.dt
    f32, f16, i16 = dt.float32, dt.float16, dt.int16
    AF = mybir.ActivationFunctionType
    OP = mybir.AluOpType
    ts = bass.ts

    natoms = ng * GA
    nidx = GA * T                  # gather indices per group (4096 for GA=8)

    nc = bacc.Bacc("TRN2", debug=False)

    rt_d = nc.dram_tensor("rt", [ng, 4, 32, GA, 128], f16, kind="ExternalInput")
    idx_d = nc.dram_tensor("idx", [ng, 2, 128, nidx // 16], i16, kind="ExternalInput")
    xT_d = nc.dram_tensor("xT", [128, A], f32, kind="ExternalInput")
    win_d = nc.dram_tensor("win", [128, F], f32, kind="ExternalInput")
    wf1_d = nc.dram_tensor("wf1r", [128, F], f16, kind="ExternalInput")
    w2t_d = nc.dram_tensor("w2t", [128, F + 1], f32, kind="ExternalInput")
    wout_d = nc.dram_tensor("wout", [128, F], f32, kind="ExternalInput")
    bout_d = nc.dram_tensor("bout", [128, 1], f32, kind="ExternalInput")
    ident_d = nc.dram_tensor("ident", [128, 128], f32, kind="ExternalInput")
    if general_mask:
        msk_d = nc.dram_tensor("msk", [ng, 128, GA, 4], f32, kind="ExternalInput")
    out_d = nc.dram_tensor("out", [natoms, F], f32, kind="ExternalOutput")
    if dbg:
        ydbg_d = nc.dram_tensor("ydbg", [128, 4, F], f16, kind="ExternalOutput")
        yjdbg_d = nc.dram_tensor("yjdbg", [128, GA * 4, F], f16, kind="ExternalOutput")
        pdbg_d = nc.dram_tensor("pdbg", [128, GA * 4, F], f16, kind="ExternalOutput")
        s0dbg_d = nc.dram_tensor("s0dbg", [128, 4, F], f16, kind="ExternalOutput")
        cdbg_d = nc.dram_tensor("cdbg", [128, F + 1], f32, kind="ExternalOutput")
        otdbg_d = nc.dram_tensor("otdbg", [128, natoms], f32, kind="ExternalOutput")

    with tile.TileContext(nc) as tc, ExitStack() as ctx:
        const = ctx.enter_context(tc.tile_pool(name="const", bufs=1))
        rt_pool = ctx.enter_context(tc.tile_pool(name="rt", bufs=3))
        gj_pool = ctx.enter_context(tc.tile_pool(name="gj", bufs=2))
        gk_pool = ctx.enter_context(tc.tile_pool(name="gk", bufs=2))
        p_pool = ctx.enter_context(tc.tile_pool(name="p", bufs=2))
        s0_pool = ctx.enter_context(tc.tile_pool(name="s0", bufs=3))
        d_pool = ctx.enter_context(tc.tile_pool(name="d", bufs=2))
        misc = ctx.enter_context(tc.tile_pool(name="misc", bufs=1))
        big_ps = ctx.enter_context(tc.tile_pool(name="bigps", bufs=1, space="PSUM"))
        sm_ps = ctx.enter_context(tc.tile_pool(name="smps", bufs=2, space="PSUM"))
        dram = ctx.enter_context(tc.tile_pool(name="dram", bufs=1, space="DRAM"))

        # ---- constants resident in SBUF
        xT = const.tile([128, A], f32)
        nc.sync.dma_start(xT[:], xT_d.ap())
        win = const.tile([128, F], f32)
        nc.sync.dma_start(win[:], win_d.ap())
        wf1r = const.tile([128, F], f16)
        nc.sync.dma_start(wf1r[:], wf1_d.ap())
        w2t = const.tile([128, F + 1], f32)
        nc.sync.dma_start(w2t[:], w2t_d.ap())
        wout = const.tile([128, F], f32)
        nc.sync.dma_start(wout[:], wout_d.ap())
        bout = const.tile([128, 1], f32)
        nc.sync.dma_start(bout[:], bout_d.ap())
        ident = const.tile([128, 128], f32)
        nc.sync.dma_start(ident[:], ident_d.ap())
        ones_col = const.tile([128, 1], f16)
        nc.vector.memset(ones_col[:], 1.0)
        neglog2 = const.tile([128, 1], f32)
        nc.vector.memset(neglog2[:], -LOG2)
        ones_f32 = const.tile([128, 1], f32)
        nc.vector.memset(ones_f32[:], 1.0)

        idx_sb = const.tile([128, ng, 2, nidx // 16], i16)
        nc.sync.dma_start(idx_sb[:], idx_d.ap().rearrange("g j p s -> p g j s"))
        if general_mask:
            msk_sb = const.tile([128, ng, GA, 4], f32)
            nc.sync.dma_start(msk_sb[:], msk_d.ap().rearrange("g p a c -> p g a c"))

        # ---- y = x @ Win, cast fp16, stage to DRAM scratch for the gathers
        ydram = dram.tile([A, F], f16)
        y_ps = big_ps.tile([128, 2048], f32, tag="bigps")
        for c in range(4):
            nc.tensor.matmul(
                y_ps[:, c * 512 : c * 512 + F],
                lhsT=xT[:, ts(c, 128)],
                rhs=win[:],
                start=True,
                stop=True,
            )
        y16 = misc.tile([128, 4, F], f16)
        nc.vector.tensor_copy(
            y16[:], y_ps[:].rearrange("p (c x) -> p c x", c=4)[:, :, 0:F]
        )
        nc.sync.dma_start(ydram.rearrange("(c p) f -> p c f", p=128), y16[:])

        # ---- main loop over gather groups / atoms
        outT = misc.tile([128, natoms], f32)
        for g in range(ng):
            rt = rt_pool.tile([128, GA, 128], f16)
            nc.sync.dma_start(rt[:], rt_d.ap()[g].rearrange("c r a p -> (c r) a p"))

            # dma_gather is limited to ~1024 descriptors per call (SWDGE
            # descriptor-ring carveout); slice the group gather.
            GMAX = 1024
            nslice = nidx // GMAX
            yj = gj_pool.tile([128, GA * 4, F], f16)
            yk = gk_pool.tile([128, GA * 4, F], f16)
            for q in range(nslice):
                nc.gpsimd.dma_gather(
                    yj[:, q * (GMAX // 128) : (q + 1) * (GMAX // 128), :],
                    ydram[:],
                    idx_sb[:, g, 0, q * (GMAX // 16) : (q + 1) * (GMAX // 16)],
                    GMAX,
                    GMAX,
                    F,
                )
                nc.gpsimd.dma_gather(
                    yk[:, q * (GMAX // 128) : (q + 1) * (GMAX // 128), :],
                    ydram[:],
                    idx_sb[:, g, 1, q * (GMAX // 16) : (q + 1) * (GMAX // 16)],
                    GMAX,
                    GMAX,
                    F,
                )

            if dbg and g == 0:
                nc.sync.dma_start(ydbg_d.ap(), y16[:])
                nc.sync.dma_start(yjdbg_d.ap(), yj[:])
            p_t = p_pool.tile([128, GA * 4, F], f16)
            if general_mask:
                for ga in range(GA):
                    for c in range(4):
                        nc.vector.scalar_tensor_tensor(
                            p_t[:, ga * 4 + c, :],
                            yj[:, ga * 4 + c, :],
                            msk_sb[:, g, ga, c : c + 1],
                            yk[:, ga * 4 + c, :],
                            op0=OP.mult,
                            op1=OP.mult,
                        )
            else:
                nc.vector.tensor_mul(
                    p_t[:].rearrange("p a f -> p (a f)"),
                    yj[:].rearrange("p a f -> p (a f)"),
                    yk[:].rearrange("p a f -> p (a f)"),
                )

            if dbg and g == 0:
                nc.sync.dma_start(pdbg_d.ap(), p_t[:])
            for ga in range(GA):
                a_loc = g * GA + ga
                h_ps = big_ps.tile([128, 2048], f32, tag="bigps")
                for c in range(4):
                    nc.tensor.matmul(
                        h_ps[:, c * 512 : c * 512 + F],
                        lhsT=rt[32 * c : 32 * c + NRBF + 1, ga, :],
                        rhs=wf1r[32 * c : 32 * c + NRBF + 1, :],
                        start=True,
                        stop=True,
                        tile_position=(32 * c, 0),
                    )
                # softplus = ln(1 + exp(.)) -- no Softplus LUT in this toolchain
                e_t = s0_pool.tile([128, 4, F], f16, tag="et")
                nc.scalar.activation(
                    e_t[:],
                    h_ps[:].rearrange("p (c x) -> p c x", c=4)[:, :, 0:F],
                    AF.Exp,
                )
                # s0 carries an appended ones column (col F) so the same
                # accumulating matmul also produces PS = sum_t P.
                s0 = s0_pool.tile([128, 4, F + 1], f16)
                nc.scalar.activation(s0[:, :, 0:F], e_t[:], AF.Ln, bias=ones_f32[:])
                nc.vector.memset(s0[:, :, F : F + 1], 1.0)
                c_ps = sm_ps.tile([128, F + 1], f32, tag="smps")
                for c in range(4):
                    nc.tensor.matmul(
                        c_ps[:],
                        lhsT=p_t[:, ga * 4 + c, :],
                        rhs=s0[:, c, :],
                        start=(c == 0),
                        stop=(c == 3),
                    )
                if dbg and g == 0 and ga == 0:
                    cdbg_s = misc.tile([128, F + 1], f32)
                    nc.vector.tensor_copy(cdbg_s[:], c_ps[:])
                    nc.sync.dma_start(cdbg_d.ap(), cdbg_s[:])
                    nc.sync.dma_start(s0dbg_d.ap(), s0[:])
                d_t = d_pool.tile([128, F + 1], f32)
                nc.vector.tensor_mul(d_t[:], c_ps[:], w2t[:])
                nc.vector.tensor_reduce(
                    outT[:, a_loc : a_loc + 1],
                    d_t[:],
                    mybir.AxisListType.X,
                    OP.add,
                )

        # ---- tail: out = softplus(out_pre @ Wout + bout) - log2
        if dbg:
            nc.sync.dma_start(otdbg_d.ap(), outT[:])
        zo_ps = sm_ps.tile([128, natoms], f32, tag="smps")
        nc.tensor.matmul(zo_ps[:], lhsT=wout[:], rhs=outT[:, 0:natoms], start=True, stop=True)
        ze = misc.tile([128, natoms], f32)
        nc.scalar.activation(ze[:], zo_ps[:], AF.Exp, bias=bout[:])
        zs = misc.tile([128, natoms], f32)
        nc.scalar.activation(zs[:], ze[:], AF.Ln, bias=ones_f32[:])
        zt_ps = sm_ps.tile([128, 128], f32, tag="smps")
        nc.tensor.transpose(zt_ps[0:natoms, :], zs[:, 0:natoms], ident[:])
        zf = misc.tile([natoms, 128], f32)
        nc.scalar.activation(zf[:], zt_ps[0:natoms, :], AF.Identity, bias=neglog2[0:natoms, :])
        nc.sync.dma_start(out_d.ap(), zf[:])

    nc.compile()
    return nc


def _wrap_idx(nbr):
    """[B, A, T] int -> per-group dma_gather index planes [B, NGG, 128, GA*T//16]."""
    v = nbr.astype(np.int16).reshape(B, NGG, GA * T)
    w = v.reshape(B, NGG, GA * T // 16, 16)
    w = np.ascontiguousarray(w.transpose(0, 1, 3, 2))      # [b, g, 16, s]
    return np.tile(w, (1, 1, 8, 1))                        # replicate to 128 parts


def prep_inputs(inputs, ng=NG_FULL, general_mask=False):
    """Full problem inputs -> list of 8 per-core input maps."""
    x = np.asarray(inputs["x"], np.float32)
    r_ij = np.asarray(inputs["r_ij"], np.float32)
    mask = np.asarray(inputs["pairwise_mask"], np.float32)
    Wf1 = np.asarray(inputs["Wf1"], np.float32)
    bf1 = np.asarray(inputs["bf1"], np.float32)
    Wf2 = np.asarray(inputs["Wf2"], np.float32)
    bf2 = np.asarray(inputs["bf2"], np.float32)
    Win = np.asarray(inputs["Win"], np.float32)
    Wout = np.asarray(inputs["Wout"], np.float32)
    bout = np.asarray(inputs["bout"], np.float32)
    nj = np.asarray(inputs["neighbors_j"])
    nk = np.asarray(inputs["neighbors_k"])

    r6 = r_ij.reshape(B, NGG, GA, 4, 128, NRBF)
    rt = np.zeros((B, NGG, 4, 32, GA, 128), np.float16)
    rt[:, :, :, :NRBF] = r6.transpose(0, 1, 3, 5, 2, 4)
    rt[:, :, :, NRBF] = 1.0

    ij = _wrap_idx(nj)
    ik = _wrap_idx(nk)
    idx = np.ascontiguousarray(np.stack([ij, ik], axis=2))  # [B, NGG, 2, 128, s]

    xT = np.ascontiguousarray(x.transpose(0, 2, 1)).astype(np.float32)

    wf1r = np.zeros((128, F), np.float16)
    wf1aug = np.vstack([Wf1, bf1[None, :]])
    for c in range(4):
        wf1r[32 * c : 32 * c + NRBF + 1] = wf1aug.astype(np.float16)

    beta = bf2 - LOG2 * Wf2.sum(axis=0)
    w2t = np.ascontiguousarray(np.vstack([Wf2, beta[None, :]]).T).astype(np.float32)

    boutc = np.ascontiguousarray(bout.reshape(F, 1)).astype(np.float32)
    ident = np.eye(128, dtype=np.float32)

    if general_mask:
        m6 = mask.reshape(B, NGG, GA, 4, 128)
        msk = np.ascontiguousarray(m6.transpose(0, 1, 4, 2, 3)).astype(np.float32)

    gpb = NGG // (CORES // B)  # groups per core (full size)
    in_maps = []
    for k in range(CORES):
        b = k // (CORES // B)
        g0 = (k % (CORES // B)) * gpb
        m = {
            "rt": np.ascontiguousarray(rt[b, g0 : g0 + ng]),
            "idx": np.ascontiguousarray(idx[b, g0 : g0 + ng]),
            "xT": xT[b],
            "win": Win.astype(np.float32),
            "wf1r": wf1r,
            "w2t": w2t,
            "wout": Wout.astype(np.float32),
            "bout": boutc,
            "ident": ident,
        }
        if general_mask:
            m["msk"] = np.ascontiguousarray(msk[b, g0 : g0 + ng])
        in_maps.append(m)
    return in_maps


def get_program(ng=NG_FULL, general_mask=False, dbg=False):
    key = (ng, general_mask, dbg)
    if key not in _programs:
        _programs[key] = _build(ng, general_mask, dbg)
    return _programs[key]


def assemble_output(results, ng=NG_FULL):
    """Per-core 'out' arrays -> full [B, A, F] float32."""
    gpb = NGG // (CORES // B)
    out = np.zeros((B, A, F), np.float32)
    for k in range(CORES):
        b = k // (CORES // B)
        g0 = (k % (CORES // B)) * gpb
        out[b, g0 * GA : g0 * GA + ng * GA] = results[k]["out"]
    return out


def kernel(**inputs) -> np.ndarray:
    from concourse import bass_utils

    mask = np.asarray(inputs["pairwise_mask"], np.float32)
    general_mask = not np.all(mask == 1.0)
    nc = get_program(NG_FULL, general_mask)
    in_maps = prep_inputs(inputs, NG_FULL, general_mask)
    res = bass_utils.run_bass_kernel_spmd(nc, in_maps, core_ids=list(range(CORES)))
    return assemble_output(res.results)


if __name__ == "__main__":
    pass



# revision 2
# speedup vs baseline: 1.0035x; 1.0035x over previous
"""CFConv-Angular (SchNet triplet message passing) on 8 Trainium2 NeuronCores.

v2: matmul-based gathers (no SWDGE dma_gather), fully transposed
[feature-partition, t-column] dataflow.

Math per (batch b, atom a), t = 0..511 triplet slots, f/g = 0..127 features:
    H[t,g]   = r_ij[t,:] @ Wf1 + bf1
    s0'[t,g] = softplus(H) - log2 = ln(0.5 + 0.5*exp(H))
    W[t,f]   = s0' @ Wf2 (+ bf2)
    y        = x @ Win                               [A, F]
    out_pre[f] = sum_t W[t,f] * y[j_t,f] * y[k_t,f] * mask[t]
    out      = ssp(out_pre @ Wout + bout)

On-device layout (per core = one (batch, atom-block) pair, 128 atoms):
    - per atom, its 512 t-slots are host-permuted so j-blocks (j>>7) are
      sorted ascending; r_ij is permuted identically (sum over t invariant).
    - HT_ps[g,t]  = Wf1aug^T @ rT          (PE, ones-row adds bf1)
    - e16 = Exp(HT), s0' = Ln(0.5*e16+0.5) (ACT, batched to avoid table swaps)
    - WT_ps[f,t]  = Wf2^T @ s0'            (PE)
    - yjT_ps[f,t] = sum_m y16_m^T @ Ej_m   (PE, 4 fixed 256-col windows)
    - ykT_ps[f,t] = sum_c y16_c^T @ Ek_c   (PE, 4 full 512-col chunks)
    - P16 = yjT*ykT                        (DVE)
    - outT[:,a], _ = TTR(P16 * WT_ps, reduce add)   (DVE fused)
    - tail: z[o,a] = Wout^T @ outT; out = relu(z) + Ln(0.5 + 0.5*Exp(-|z|))

E matrices are host-built one-hot (mask folded into Ej), fp8e4 (0/1 exact).
"""

import os
import sys
from contextlib import ExitStack

import numpy as np

for _p in ("/opt/trn_rl_repo", "/root/.axon_site/_ro/trn_rl_repo"):
    if os.path.isdir(_p) and _p not in sys.path:
        sys.path.append(_p)

B, A, T, NRBF, F = 2, 512, 512, 25, 128
CORES = 8
APC = 128                      # atoms per core
NB = 8                         # atoms per DMA batch
NBATCH = APC // NB             # 16 batches per core
WJ = 256                       # j-gather window width (blocks 1..3)
WSTART = (0, 64, 192, 256)     # per-block window starts
# j-gather windows in ISSUE order: (block, start, width). The first is
# full-width with start=True so later windows accumulate into an
# already-started PSUM range (walrus rejects accumulation onto ranges no
# start covered; sim zeroes whole banks on start).
WSPEC = ((0, 0, 512), (3, 256, 256), (1, 64, 256), (2, 192, 256))
WSPEC_WIDE = tuple((m, 0, 512) for m in range(4))
NJC = sum(w for _, _, w in WSPEC)        # ej cols per atom (1280)
LOG2 = float(np.log(2.0))

_programs = {}


def _build(nbatch=NBATCH, dbg=False, wide_j=False):
    import concourse.bacc as bacc
    import concourse.bass as bass
    import concourse.tile as tile
    from concourse import mybir

    dt = mybir.dt
    f32, f16, f8 = dt.float32, dt.float16, dt.float8e4
    AF = mybir.ActivationFunctionType
    OP = mybir.AluOpType

    natoms = nbatch * NB
    wspec = WSPEC_WIDE if wide_j else WSPEC
    njc = sum(w for _, _, w in wspec)

    nc = bacc.Bacc("TRN2", debug=False)

    xT_d = nc.dram_tensor("xT", [128, A], f16, kind="ExternalInput")
    win_d = nc.dram_tensor("win", [128, F], f16, kind="ExternalInput")
    wf1_d = nc.dram_tensor("wf1", [NRBF + 1, F], f16, kind="ExternalInput")
    wf2_d = nc.dram_tensor("wf2", [128, F], f16, kind="ExternalInput")
    wout_d = nc.dram_tensor("wout", [128, F], f32, kind="ExternalInput")
    bout_d = nc.dram_tensor("bout", [128, 1], f32, kind="ExternalInput")
    rt_d = nc.dram_tensor("rt", [nbatch, NRBF + 1, NB * T], f16, kind="ExternalInput")
    ej_d = nc.dram_tensor("ej", [nbatch, 128, NB * njc], f8, kind="ExternalInput")
    ek_d = nc.dram_tensor("ek", [nbatch, 128, NB * 4 * T], f8, kind="ExternalInput")
    out_d = nc.dram_tensor("out", [128, natoms], f32, kind="ExternalOutput")
    if dbg:
        y16dbg_d = nc.dram_tensor("y16dbg", [128, A], f16, kind="ExternalOutput")
        s0dbg_d = nc.dram_tensor("s0dbg", [128, T], f16, kind="ExternalOutput")
        wtdbg_d = nc.dram_tensor("wtdbg", [128, T], f32, kind="ExternalOutput")
        yjdbg_d = nc.dram_tensor("yjdbg", [128, T], f32, kind="ExternalOutput")
        ykdbg_d = nc.dram_tensor("ykdbg", [128, T], f32, kind="ExternalOutput")
        otdbg_d = nc.dram_tensor("otdbg", [128, natoms], f32, kind="ExternalOutput")

    with tile.TileContext(nc) as tc, ExitStack() as ctx:
        const = ctx.enter_context(tc.tile_pool(name="const", bufs=1))
        rt_pool = ctx.enter_context(tc.tile_pool(name="rt", bufs=3))
        ej_pool = ctx.enter_context(tc.tile_pool(name="ej", bufs=3))
        ek_pool = ctx.enter_context(tc.tile_pool(name="ek", bufs=3))
        e16_pool = ctx.enter_context(tc.tile_pool(name="e16", bufs=2))
        s0_pool = ctx.enter_context(tc.tile_pool(name="s0", bufs=2))
        p_pool = ctx.enter_context(tc.tile_pool(name="p16", bufs=2))
        junk_pool = ctx.enter_context(tc.tile_pool(name="junk", bufs=2))
        misc = ctx.enter_context(tc.tile_pool(name="misc", bufs=1))
        hp = ctx.enter_context(tc.tile_pool(name="hp", bufs=2, space="PSUM"))
        wp = ctx.enter_context(tc.tile_pool(name="wp", bufs=2, space="PSUM"))
        jp = ctx.enter_context(tc.tile_pool(name="jp", bufs=2, space="PSUM"))
        kp = ctx.enter_context(tc.tile_pool(name="kp", bufs=2, space="PSUM"))

        # ---- constants
        xT = const.tile([128, A], f16)
        nc.sync.dma_start(xT[:], xT_d.ap())
        win = const.tile([128, F], f16)
        nc.sync.dma_start(win[:], win_d.ap())
        wf1 = const.tile([NRBF + 1, F], f16)
        nc.sync.dma_start(wf1[:], wf1_d.ap())
        wf2 = const.tile([128, F], f16)
        nc.sync.dma_start(wf2[:], wf2_d.ap())
        wout = const.tile([128, F], f32)
        nc.sync.dma_start(wout[:], wout_d.ap())
        bout = const.tile([128, 1], f32)
        nc.sync.dma_start(bout[:], bout_d.ap())
        half = const.tile([128, 1], f32)
        nc.vector.memset(half[:], 0.5)

        # ---- y = x @ Win  -> y16 [a_local(128), 4 blocks, F] fp16
        y_ps = hp.tile([128, 512], f32, tag="h_ps")
        for c in range(4):
            nc.tensor.matmul(
                y_ps[:, c * F : (c + 1) * F],
                lhsT=xT[:, bass.ts(c, 128)],
                rhs=win[:],
                start=(c == 0),
                stop=(c == 3),
                skip_group_check=True,
            )
        y16 = misc.tile([128, 512], f16)
        nc.vector.tensor_copy(y16[:], y_ps[:])
        if dbg:
            nc.sync.dma_start(y16dbg_d.ap(), y16[:])

        outT = misc.tile([128, natoms], f32)

        # ---- main loop: batch pairs for act-table amortization
        for bp in range((nbatch + 1) // 2):
            gs = [g for g in (2 * bp, 2 * bp + 1) if g < nbatch]
            tiles = {}
            for g in gs:
                rt = rt_pool.tile([NRBF + 1, NB * T], f16)
                nc.sync.dma_start(rt[:], rt_d.ap()[g])
                ej = ej_pool.tile([128, NB * njc], f8)
                nc.sync.dma_start(ej[:], ej_d.ap()[g])
                ek = ek_pool.tile([128, NB * 4 * T], f8)
                nc.sync.dma_start(ek[:], ek_d.ap()[g])
                e16 = e16_pool.tile([128, NB * T], f16)
                h_tiles = []
                for i in range(NB):
                    h_ps = hp.tile([128, T], f32)
                    nc.tensor.matmul(
                        h_ps[:],
                        lhsT=wf1[:],
                        rhs=rt[:, i * T : (i + 1) * T],
                        start=True,
                        stop=True,
                    )
                    nc.scalar.activation(e16[:, i * T : (i + 1) * T], h_ps[:], AF.Exp)
                    h_tiles.append(h_ps)
                tiles[g] = (ej, ek, e16)
            for g in gs:
                ej, ek, e16 = tiles[g]
                s0 = s0_pool.tile([128, NB * T], f16)
                # ln(0.5 + 0.5*e^H) = softplus(H) - log2
                nc.scalar.activation(s0[:], e16[:], AF.Ln, bias=half[:], scale=0.5)
                if dbg and g == 0:
                    nc.sync.dma_start(s0dbg_d.ap(), s0[:, 0:T])
                for i in range(NB):
                    a_loc = g * NB + i
                    w_ps = wp.tile([128, T], f32)
                    nc.tensor.matmul(
                        w_ps[:],
                        lhsT=wf2[:],
                        rhs=s0[:, i * T : (i + 1) * T],
                        start=True,
                        stop=True,
                    )
                    yj_ps = jp.tile([128, T], f32)
                    off = 0
                    for n, (m, st, w) in enumerate(wspec):
                        nc.tensor.matmul(
                            yj_ps[:, st : st + w],
                            lhsT=y16[:, m * F : (m + 1) * F],
                            rhs=ej[:, i * njc + off : i * njc + off + w],
                            start=(n == 0),
                            stop=(n == 3),
                            skip_group_check=True,
                        )
                        off += w
                    yk_ps = kp.tile([128, T], f32)
                    for c in range(4):
                        nc.tensor.matmul(
                            yk_ps[:],
                            lhsT=y16[:, c * F : (c + 1) * F],
                            rhs=ek[:, (i * 4 + c) * T : (i * 4 + c + 1) * T],
                            start=(c == 0),
                            stop=(c == 3),
                        )
                    if dbg and a_loc == 0:
                        wtdbg_s = misc.tile([128, T], f32)
                        nc.vector.tensor_copy(wtdbg_s[:], w_ps[:])
                        nc.sync.dma_start(wtdbg_d.ap(), wtdbg_s[:])
                        yjdbg_s = misc.tile([128, T], f32)
                        nc.vector.tensor_copy(yjdbg_s[:], yj_ps[:])
                        nc.sync.dma_start(yjdbg_d.ap(), yjdbg_s[:])
                        ykdbg_s = misc.tile([128, T], f32)
                        nc.vector.tensor_copy(ykdbg_s[:], yk_ps[:])
                        nc.sync.dma_start(ykdbg_d.ap(), ykdbg_s[:])
                    # One engine pass per PSUM tensor: ACT evacuates W
                    # (Copy is in every act table), DVE does the two
                    # products; reduce is batched below.
                    w16 = p_pool.tile([128, T], f16, tag="w16")
                    nc.vector.tensor_scalar_mul(w16[:], w_ps[:], 1.0)
                    x16 = p_pool.tile([128, T], f16, tag="x16")
                    nc.vector.tensor_mul(x16[:], w16[:], yj_ps[:])
                    q16 = junk_pool.tile([128, T], f16)
                    nc.vector.tensor_mul(q16[:], x16[:], yk_ps[:])
                    # reduce on the ACT accumulator: pipeline sink, and the
                    # Identity func is in every act table (no table load)
                    junk = junk_pool.tile([128, T], f16, tag="ajunk")
                    nc.scalar.activation(
                        junk[:],
                        q16[:],
                        AF.Identity,
                        accum_out=outT[:, a_loc : a_loc + 1],
                    )

        # ---- tail: out[o, a] = ssp(Wout^T @ outT + bout)
        if dbg:
            nc.sync.dma_start(otdbg_d.ap(), outT[:])
        z_ps = wp.tile([128, T], f32, tag="w_ps")
        nc.tensor.matmul(
            z_ps[:, 0:natoms], lhsT=wout[:], rhs=outT[:], start=True, stop=True
        )
        # overflow-safe: ssp(v) = relu(v) + ln(0.5 + 0.5*exp(-|v|)) + bout folded
        zb = misc.tile([128, natoms], f32)
        nc.scalar.activation(zb[:], z_ps[:, 0:natoms], AF.Identity, bias=bout[:])
        za = misc.tile([128, natoms], f32)
        nc.scalar.activation(za[:], zb[:], AF.Abs)
        ze = misc.tile([128, natoms], f32)
        nc.scalar.activation(ze[:], za[:], AF.Exp, scale=-1.0)
        zl = misc.tile([128, natoms], f32)
        nc.scalar.activation(zl[:], ze[:], AF.Ln, bias=half[:], scale=0.5)
        zr = misc.tile([128, natoms], f32)
        nc.vector.tensor_scalar_max(zr[:], zb[:], 0.0)
        zo = misc.tile([128, natoms], f32)
        nc.vector.tensor_add(zo[:], zl[:], zr[:])
        nc.sync.dma_start(out_d.ap()[:, 0:natoms], zo[:])

    nc.compile()
    return nc


def get_program(nbatch=NBATCH, dbg=False, wide_j=False):
    key = (nbatch, dbg, wide_j)
    if key not in _programs:
        _programs[key] = _build(nbatch, dbg, wide_j)
    return _programs[key]


def check_windows(inputs, nbatch=NBATCH):
    """True if every per-atom sorted j-run fits its fixed window."""
    nj = np.asarray(inputs["neighbors_j"]).astype(np.int64)
    natoms = nbatch * NB
    for core in range(CORES):
        b, q = core // 4, core % 4
        j = nj[b, q * APC : q * APC + natoms]
        order = np.argsort(j >> 7, axis=1, kind="stable")
        jb = j[np.arange(natoms)[:, None], order] >> 7
        tpos = np.broadcast_to(np.arange(T)[None, :], jb.shape)
        for m in range(4):
            sel = jb == m
            any_ = sel.any(1)
            lo = np.where(sel, tpos, T).min(1)
            hi = np.where(sel, tpos, -1).max(1)
            if (any_ & ((lo < WSTART[m]) | (hi >= WSTART[m] + WJ))).any():
                return False
    return True


def prep_inputs(inputs, nbatch=NBATCH, wide_j=False):
    """Full problem inputs -> list of 8 per-core input maps."""
    from concourse import mybir

    f8np = mybir.dt.np(mybir.dt.float8e4)

    x = np.asarray(inputs["x"], np.float32)
    r_ij = np.asarray(inputs["r_ij"], np.float32)
    mask = np.asarray(inputs["pairwise_mask"], np.float32)
    Wf1 = np.asarray(inputs["Wf1"], np.float32)
    bf1 = np.asarray(inputs["bf1"], np.float32)
    Wf2 = np.asarray(inputs["Wf2"], np.float32)
    bf2 = np.asarray(inputs["bf2"], np.float32)
    Win = np.asarray(inputs["Win"], np.float32)
    Wout = np.asarray(inputs["Wout"], np.float32)
    bout = np.asarray(inputs["bout"], np.float32)
    nj = np.asarray(inputs["neighbors_j"]).astype(np.int64)
    nk = np.asarray(inputs["neighbors_k"]).astype(np.int64)

    assert np.allclose(bf2, 0.0), "bf2 != 0 not supported by this kernel"

    wf1aug = np.vstack([Wf1, bf1[None, :]]).astype(np.float16)      # [26, F]
    wf2_16 = Wf2.astype(np.float16)                                  # [g, f]
    wout32 = Wout.astype(np.float32)                                 # [f, o]

    wspec = WSPEC_WIDE if wide_j else WSPEC
    njc = sum(w for _, _, w in wspec)
    # per-block (start, col-offset within atom's ej region), in block order
    woff = {}
    off = 0
    for m, st, w in wspec:
        woff[m] = (st, off)
        off += w
    wstart_arr = np.asarray([woff[m][0] for m in range(4)])
    wcoff_arr = np.asarray([woff[m][1] for m in range(4)])

    natoms = nbatch * NB
    in_maps = []
    for core in range(CORES):
        b = core // 4
        q = core % 4
        a0 = q * APC
        # per-atom j-block sort permutation
        j = nj[b, a0 : a0 + natoms]                     # [na, T]
        k = nk[b, a0 : a0 + natoms]
        order = np.argsort(j >> 7, axis=1, kind="stable")
        ai = np.arange(natoms)[:, None]
        jp_ = j[ai, order]                              # [na, T]
        kp_ = k[ai, order]
        mp_ = mask[b, a0 : a0 + natoms][ai, order]
        rp_ = r_ij[b, a0 : a0 + natoms][ai, order]      # [na, T, NRBF]

        tpos = np.broadcast_to(np.arange(T)[None, :], jp_.shape)

        # rt: [nbatch, 26, NB*T] fp16 with ones row
        rt = np.empty((nbatch, NRBF + 1, NB * T), np.float16)
        rfull = rp_.transpose(0, 2, 1).reshape(nbatch, NB, NRBF, T)
        rt[:, :NRBF] = rfull.transpose(0, 2, 1, 3).reshape(nbatch, NRBF, NB * T)
        rt[:, NRBF] = 1.0

        # Ej: [nbatch, 128, NB*njc]; window of block m at col-offset woff[m]
        ej = np.zeros((natoms, 128, njc), np.float32)
        am = np.repeat(np.arange(natoms), T)
        jf = jp_.reshape(-1)
        mf = mp_.reshape(-1)
        tf = tpos.reshape(-1)
        blk = jf >> 7
        col = wcoff_arr[blk] + (tf - wstart_arr[blk])
        ej[am, jf & 127, col] = mf
        ej = (
            ej.reshape(nbatch, NB, 128, njc)
            .transpose(0, 2, 1, 3)
            .reshape(nbatch, 128, NB * njc)
            .astype(f8np)
        )

        # Ek: [nbatch, 128, NB*4*T]; chunk c of atom => cols [(i*4+c)*T ...)
        ek = np.zeros((natoms, 128, 4 * T), np.float32)
        kf = kp_.reshape(-1)
        colk = (kf >> 7) * T + tf
        ek[am, kf & 127, colk] = 1.0
        ek = (
            ek.reshape(nbatch, NB, 128, 4 * T)
            .transpose(0, 2, 1, 3)
            .reshape(nbatch, 128, NB * 4 * T)
            .astype(f8np)
        )

        m = {
            "xT": np.ascontiguousarray(x[b].T).astype(np.float16),
            "win": Win.astype(np.float16),
            "wf1": wf1aug,
            "wf2": wf2_16,
            "wout": wout32,
            "bout": bout.reshape(F, 1).astype(np.float32),
            "rt": np.ascontiguousarray(rt),
            "ej": np.ascontiguousarray(ej),
            "ek": np.ascontiguousarray(ek),
        }
        in_maps.append(m)
    return in_maps


def assemble_output(results, nbatch=NBATCH):
    natoms = nbatch * NB
    out = np.zeros((B, A, F), np.float32)
    for core in range(CORES):
        b = core // 4
        a0 = (core % 4) * APC
        out[b, a0 : a0 + natoms] = results[core]["out"][:, 0:natoms].T
    return out


def kernel(**inputs) -> np.ndarray:
    from concourse import bass_utils

    wide_j = not check_windows(inputs, NBATCH)
    nc = get_program(NBATCH, wide_j=wide_j)
    in_maps = prep_inputs(inputs, NBATCH, wide_j=wide_j)
    res = bass_utils.run_bass_kernel_spmd(nc, in_maps, core_ids=list(range(CORES)))
    return assemble_output(res.results)


if __name__ == "__main__":
    pass
